# revision 1
# baseline (speedup 1.0000x reference)
"""Trainium2 Bass kernel: 4096x4096 fp32 'valid' cross-correlation with a 15x15
kernel, plus scalar bias.

Strategy
--------
- Shard the output 2x4 across 8 NeuronCores: 2 W-stripes of 2048 cols x 4
  H-bands of 1026 rows (4x1026 >= 4082; tails trimmed on the host). Each
  core's input is its stripe/band plus a 14-pixel halo on each axis, gathered
  on the host from a zero-padded copy -- no device-to-device communication.
  The wide stripes make every DMA ~0.5 MB (4 KB/partition), near line rate;
  the old 512-col stripes moved 134 KB per DMA at ~30% efficiency.
- Per core, 9 h-chunks of 114 output rows: a [K<=128, 114] banded-Toeplitz
  stationary (T_dj[k, m] = weight[k-m, dj]) contracts 128 input rows against
  114 output rows; the W-shift for dj is a free-dim offset in the moving
  operand (image rows in SBUF partitions, W along free). 15 dj passes x 4
  512-col blocks per chunk accumulate in PSUM.
- bf16 operands (1 cycle/row on the PE vs 4 for fp32; fp32 PSUM accumulation
  keeps rel err ~3e-3 << 2e-2), bf16 output (halves store traffic vs fp32;
  upcast on host).
- Input DMAs issue on the SP HWDGE ring (nc.sync), output DMAs on the ACT
  ring (nc.scalar) so loads and stores overlap instead of sharing one FIFO.
"""

import numpy as np

H, W = 4096, 4096
KH, KW = 15, 15
HO, WO = H - KH + 1, W - KW + 1  # 4082, 4082
NCORES = 8
WSH, HSH = 2, 4          # core grid: 2 W-stripes x 4 H-bands
C = 2048                 # output cols per stripe
CIN = C + KW - 1         # input cols per stripe (with halo) = 2062
MCH = 114                # output rows per h-chunk (114 + 14 = 128 = K)
NCHUNK = 9               # chunks per band
B = NCHUNK * MCH         # output rows per band = 1026
BIN = B + KH - 1         # input rows per band = 1040
NBLK = C // 512          # 512-col psum blocks per chunk
XR_PAD = HSH * B + KH - 1   # padded input rows = 4118
XC_PAD = WSH * C + KW - 1   # padded input cols = 4110

_CACHE = {}


def _bf16():
    import ml_dtypes
    return ml_dtypes.bfloat16


def _enable_ldw_opt():
    """Flip walrus --enable-ldw-opt to true (dedupes identical consecutive
    weight loads, which the dj-outer/block-inner schedule produces)."""
    import concourse.bass_utils as bu
    if getattr(bu.run_command, "_ldw_patched", False):
        return
    orig = bu.run_command

    def patched(argv, **kw):
        argv = ["--enable-ldw-opt=true" if a == "--enable-ldw-opt=false" else a
                for a in argv]
        return orig(argv, **kw)

    patched._ldw_patched = True
    bu.run_command = patched


def _build_nc(reps: int = 1, n_dj: int = KW, hw_loop: bool = False,
              schedule: str = "dj_inner", pp_bufs: int = 6, xp_bufs: int = 2,
              blk_w: int = 512,
              parts: tuple = ("in", "mm", "drain", "out")):
    import concourse.bacc as bacc
    import concourse.mybir as mybir
    from concourse.tile import TileContext

    parts = set(parts)
    f32 = mybir.dt.float32
    bf16 = mybir.dt.bfloat16

    nc = bacc.Bacc("TRN2", debug=False, num_devices=NCORES)
    xs_d = nc.dram_tensor("xs", [BIN, CIN], bf16, kind="ExternalInput")
    wT_d = nc.dram_tensor("wT", [128, KW, 128], bf16, kind="ExternalInput")
    bias_d = nc.dram_tensor("bias", [1, 1], f32, kind="ExternalInput")
    ys_d = nc.dram_tensor("ys", [B, C], bf16, kind="ExternalOutput")

    with TileContext(nc) as tc:
        with (
            tc.tile_pool(name="xp", bufs=xp_bufs) as xp,
            tc.tile_pool(name="wp", bufs=1) as wp,
            tc.tile_pool(name="op", bufs=3) as op,
            tc.tile_pool(name="pp", bufs=(pp_bufs if blk_w == 512 else 3)
                         if schedule == "dj_inner" else 2,
                         space="PSUM") as pp,
        ):
            # Weights (Toeplitz stack, M padded to 128 cols for FWL) + bias
            w_t = wp.tile([128, KW, 128], bf16)
            nc.sync.dma_start(w_t[:, :, :], wT_d[:, :, :])
            bias_t = wp.tile([1, 1], f32)
            nc.sync.dma_start(bias_t[:, :], bias_d[:, :])
            bias_bc = wp.tile([128, 1], f32)
            nc.gpsimd.partition_broadcast(bias_bc[:, :], bias_t[:, :])

            # Static stand-ins for isolated-stage probe builds
            x_s = o_s = None
            if "mm" in parts and "in" not in parts:
                x_s = wp.tile([128, CIN], bf16)
                nc.sync.dma_start(x_s[:, :], xs_d[0:128, :])
            if "out" in parts and "drain" not in parts:
                o_s = wp.tile([MCH, C], bf16)
                nc.vector.memset(o_s[:, :], 0.0)

            def rep_body(_i=None):
                for ci in range(NCHUNK):
                    m0 = ci * MCH
                    if "in" in parts:
                        x_b = xp.tile([128, CIN], bf16, name="x_b")
                        nc.sync.dma_start(x_b[:, :], xs_d[m0:m0 + 128, :])
                    else:
                        x_b = x_s
                    if "drain" in parts:
                        o = op.tile([MCH, C], bf16, name="o")
                    else:
                        o = o_s
                    if "mm" in parts and schedule == "dj_inner":
                        for blk in range(C // blk_w):
                            j0 = blk * blk_w
                            ps = pp.tile([128, blk_w], f32, name="ps")
                            for dj in range(n_dj):
                                nc.tensor.matmul(
                                    ps[:, :],
                                    w_t[:, dj, :],
                                    x_b[:, j0 + dj:j0 + dj + blk_w],
                                    start=(dj == 0),
                                    stop=(dj == n_dj - 1),
                                )
                            if "drain" in parts:
                                nc.vector.tensor_scalar_add(
                                    o[:, j0:j0 + blk_w],
                                    ps[0:MCH, :],
                                    bias_bc[0:MCH, 0:1],
                                )
                    elif "mm" in parts:
                        # dj-outer/block-inner: 4 consecutive matmuls share
                        # one stationary (deduped by ldw-opt); 4 psum banks
                        # accumulate interleaved.
                        pss = [pp.tile([128, 512], f32, name=f"ps{b}")
                               for b in range(NBLK)]
                        for dj in range(n_dj):
                            for blk in range(NBLK):
                                j0 = blk * 512 + dj
                                nc.tensor.matmul(
                                    pss[blk][:, :],
                                    w_t[:, dj, :],
                                    x_b[:, j0:j0 + 512],
                                    start=(dj == 0),
                                    stop=(dj == n_dj - 1),
                                    skip_group_check=True,
                                )
                        if "drain" in parts:
                            for blk in range(NBLK):
                                nc.vector.tensor_scalar_add(
                                    o[:, blk * 512:(blk + 1) * 512],
                                    pss[blk][0:MCH, :],
                                    bias_bc[0:MCH, 0:1],
                                )
                    if "out" in parts:
                        nc.scalar.dma_start(
                            ys_d[m0:m0 + MCH, :], o[:, :]
                        )

            if hw_loop and reps > 1:
                tc.For_i_unrolled(0, reps, 1, rep_body, max_unroll=8)
            else:
                for _rep in range(reps):
                    rep_body()

    nc.compile()
    return nc


def _toeplitz_stack(weight: np.ndarray) -> np.ndarray:
    """wT[k, dj, m] = weight[k-m, dj] for 0 <= k-m < KH (m < MCH; cols
    MCH..127 are zero padding so LDWEIGHTS uses the fast-weight-load path)."""
    wT = np.zeros((128, KW, 128), dtype=np.float32)
    for di in range(KH):
        for m in range(MCH):
            wT[m + di, :, m] = weight[di, :]
    return wT


def _prepare_in_maps(x, weight, bias):
    bf16 = _bf16()
    x = np.ascontiguousarray(x, dtype=np.float32)
    weight = np.asarray(weight, dtype=np.float32)
    bias_v = np.asarray(bias, dtype=np.float32).reshape(-1)[:1]

    x_pad = np.zeros((XR_PAD, XC_PAD), dtype=np.float32)
    x_pad[:H, :W] = x
    x_pad = x_pad.astype(bf16)
    wT = _toeplitz_stack(weight).astype(bf16)
    bias_in = bias_v.reshape(1, 1)

    in_maps = []
    for core in range(NCORES):
        c, r = core // HSH, core % HSH
        xs = x_pad[r * B:r * B + BIN, c * C:c * C + CIN]
        in_maps.append(
            {"xs": np.ascontiguousarray(xs), "wT": wT, "bias": bias_in}
        )
    return in_maps


def kernel(x: np.ndarray, weight: np.ndarray, bias: np.ndarray) -> np.ndarray:
    from concourse.bass_utils import run_bass_kernel_spmd

    if "nc" not in _CACHE:
        _CACHE["nc"] = _build_nc()
    nc = _CACHE["nc"]

    in_maps = _prepare_in_maps(x, weight, bias)
    res = run_bass_kernel_spmd(nc, in_maps, core_ids=list(range(NCORES)))

    out = np.empty((HO, WO), dtype=np.float32)
    for core in range(NCORES):
        c, r = core // HSH, core % HSH
        r0, r1 = r * B, min(r * B + B, HO)
        c0, c1 = c * C, min(c * C + C, WO)
        ys = res.results[core]["ys"]
        out[r0:r1, c0:c1] = ys[: r1 - r0, : c1 - c0].astype(np.float32)
    return out



# revision 3
# speedup vs baseline: 1.3256x; 1.3256x over previous
"""Trainium2 Bass kernel: 4096x4096 fp32 'valid' cross-correlation with a 15x15
kernel, plus scalar bias.

Strategy (v2: fp8 DoubleRow)
----------------------------
- Shard the output 2x4 across 8 NeuronCores: 2 W-stripes of 2048 cols x 4
  H-bands of 1026 rows (tails trimmed on the host). Per core, 9 h-chunks of
  114 output rows: a banded-Toeplitz stationary (T[k, m] = wcol[k-m])
  contracts 128 input rows against 114 output rows.
- fp8e4m3 operands with perf_mode=DoubleRow: the PE virtualizes to 128x256
  (2 fp8 weights/cell, 2 MACs/cycle, 0.5 cycles per moving column). The two
  interleave planes carry the SAME image rows at column shifts (0, +8), so
  one pass computes TWO column-taps (t, t+8). The 15x15 kernel is split
  w = w_hi + w_lo (both e4m3; residual ~1e-3) giving 29 tap-slots on device
  (w_lo column 7 is folded into the host correction) = 15 DoubleRow passes
  per chunk vs 15 bf16 passes at 1 cycle/column in v1 -> ~2x PE time.
- Accuracy (rel-err budget 2e-2): device error is dominated by fp8
  quantization of x. The dominant component is mean(w) * boxsum(x - xq),
  which the host computes exactly via prefix sums and uploads as a bf16
  correction field C (bias folded in); the drain adds C on the DVE. The
  remaining variance-floor error measures ~1.4e-2 rel.
- Input DMAs on the SP ring (nc.sync), C loads on the GPSIMD ring, output
  stores on the ACT ring (nc.scalar) so the streams don't share one FIFO.
"""

import numpy as np

H, W = 4096, 4096
KH, KW = 15, 15
HO, WO = H - KH + 1, W - KW + 1  # 4082, 4082
NCORES = 8
WSH, HSH = 2, 4          # core grid: 2 W-stripes x 4 H-bands
C = 2048                 # output cols per stripe
MCH = 114                # output rows per h-chunk (114 + 14 = 128 = K)
NCHUNK = 9               # chunks per band
B = NCHUNK * MCH         # output rows per band = 1026
BIN = B + KH - 1         # input rows per band = 1040
NBLK = C // 512          # 512-col psum blocks per chunk
CW = 2064                # moving-tile plane width (mult of 16 for DoubleRow AP)
XSW = 2080               # per-core input dram width (plane1 needs cols 8..2071)
XR_PAD = HSH * B + KH - 1   # padded input rows = 4118
XC_PAD = WSH * C + XSW - C  # padded input cols = 2048 + 2080 = 4128
NPASS = 15
# pass p -> (base column shift, plane0 tap source, plane1 tap source)
# sources: ('hi', col) / ('lo', col) / None; plane1 reads at base+8.
PASS_TABLE = (
    [(t, ("hi", t), ("hi", t + 8)) for t in range(7)]
    + [(t, ("lo", t), ("lo", t + 8)) for t in range(7)]
    + [(7, ("hi", 7), None)]
)

_CACHE = {}


def _bf16():
    import ml_dtypes
    return ml_dtypes.bfloat16


def _fp8():
    import ml_dtypes
    return ml_dtypes.float8_e4m3


def _enable_ldw_opt():
    """Flip walrus --enable-ldw-opt to true (dedupes identical consecutive
    weight loads, which the pass-outer/block-inner schedule produces)."""
    import concourse.bass_utils as bu
    if getattr(bu.run_command, "_ldw_patched", False):
        return
    orig = bu.run_command

    def patched(argv, **kw):
        argv = ["--enable-ldw-opt=true" if a == "--enable-ldw-opt=false" else a
                for a in argv]
        return orig(argv, **kw)

    patched._ldw_patched = True
    bu.run_command = patched


def _build_nc(reps: int = 1, hw_loop: bool = False, ldw_opt: bool = False,
              parts: tuple = ("in", "mm", "drain", "out")):
    import concourse.bacc as bacc
    import concourse.mybir as mybir
    from concourse.tile import TileContext

    if ldw_opt:
        _enable_ldw_opt()
    parts = set(parts)
    f32 = mybir.dt.float32
    bf16 = mybir.dt.bfloat16
    fp8 = mybir.dt.float8e4

    nc = bacc.Bacc("TRN2", debug=False, num_devices=NCORES)
    xs_d = nc.dram_tensor("xs", [BIN, XSW], fp8, kind="ExternalInput")
    wT_d = nc.dram_tensor("wT", [128, NPASS, 2, 128], fp8, kind="ExternalInput")
    corr_d = nc.dram_tensor("corr", [B, C], bf16, kind="ExternalInput")
    ys_d = nc.dram_tensor("ys", [B, C], bf16, kind="ExternalOutput")

    with TileContext(nc) as tc:
        with (
            tc.tile_pool(name="xp", bufs=2) as xp,
            tc.tile_pool(name="wp", bufs=1) as wp,
            tc.tile_pool(name="cp", bufs=2) as cp,
            tc.tile_pool(name="op", bufs=3) as op,
            tc.tile_pool(name="pp", bufs=2, space="PSUM") as pp,
        ):
            w_t = wp.tile([128, NPASS, 2, 128], fp8)
            nc.sync.dma_start(w_t[:, :, :, :], wT_d[:, :, :, :])

            x_s = c_s = o_s = None
            if "mm" in parts and "in" not in parts:
                x_s = wp.tile([128, 2, CW], fp8)
                nc.sync.dma_start(x_s[:, 0, :], xs_d[0:128, 0:CW])
                nc.sync.dma_start(x_s[:, 1, :], xs_d[0:128, 8:8 + CW])
                c_s = wp.tile([MCH, C], bf16)
                nc.gpsimd.dma_start(c_s[:, :], corr_d[0:MCH, :])
            if "out" in parts and "drain" not in parts:
                o_s = wp.tile([MCH, C], bf16)
                nc.vector.memset(o_s[:, :], 0.0)

            def rep_body(_i=None):
                for ci in range(NCHUNK):
                    m0 = ci * MCH
                    if "in" in parts:
                        x_b = xp.tile([128, 2, CW], fp8, name="x_b")
                        nc.sync.dma_start(x_b[:, 0, :],
                                          xs_d[m0:m0 + 128, 0:CW])
                        nc.sync.dma_start(x_b[:, 1, :],
                                          xs_d[m0:m0 + 128, 8:8 + CW])
                        c_t = cp.tile([MCH, C], bf16, name="c_t")
                        nc.gpsimd.dma_start(c_t[:, :],
                                            corr_d[m0:m0 + MCH, :])
                    else:
                        x_b, c_t = x_s, c_s
                    if "drain" in parts:
                        o = op.tile([MCH, C], bf16, name="o")
                    else:
                        o = o_s
                    if "mm" in parts:
                        pss = [pp.tile([128, 512], f32, name=f"ps{b}")
                               for b in range(NBLK)]
                        for p, (base, _s0, _s1) in enumerate(PASS_TABLE):
                            for blk in range(NBLK):
                                j0 = blk * 512 + base
                                nc.tensor.matmul(
                                    pss[blk][:, :],
                                    w_t[:, p, :, :],
                                    x_b[:, :, j0:j0 + 512],
                                    start=(p == 0),
                                    stop=(p == NPASS - 1),
                                    perf_mode=mybir.MatmulPerfMode.DoubleRow,
                                    skip_group_check=True,
                                )
                        if "drain" in parts:
                            for blk in range(NBLK):
                                nc.vector.scalar_tensor_tensor(
                                    o[:, blk * 512:(blk + 1) * 512],
                                    pss[blk][0:MCH, :],
                                    1.0,
                                    c_t[:, blk * 512:(blk + 1) * 512],
                                    mybir.AluOpType.mult,
                                    mybir.AluOpType.add,
                                )
                    if "out" in parts:
                        nc.scalar.dma_start(ys_d[m0:m0 + MCH, :], o[:, :])

            if hw_loop and reps > 1:
                tc.For_i_unrolled(0, reps, 1, rep_body, max_unroll=8)
            else:
                for _rep in range(reps):
                    rep_body()

    nc.compile()
    return nc


def _toeplitz(col: np.ndarray) -> np.ndarray:
    """T[k, m] = col[k-m] for 0 <= k-m < KH (m < MCH; cols MCH..127 zero)."""
    T = np.zeros((128, 128), dtype=np.float32)
    for di in range(KH):
        for m in range(MCH):
            T[m + di, m] = col[di]
    return T


def _weight_stack(w_hi: np.ndarray, w_lo: np.ndarray) -> np.ndarray:
    """wT[k, pass, plane, m] per PASS_TABLE."""
    src = {"hi": w_hi, "lo": w_lo}
    wT = np.zeros((128, NPASS, 2, 128), dtype=np.float32)
    for p, (_base, s0, s1) in enumerate(PASS_TABLE):
        for plane, s in enumerate((s0, s1)):
            if s is not None:
                wT[:, p, plane, :] = _toeplitz(src[s[0]][:, s[1]])
    return wT


def _boxsum15(a: np.ndarray) -> np.ndarray:
    """Valid 15x15 box sum (fp64 prefix sums)."""
    c = np.cumsum(np.cumsum(a, axis=0, dtype=np.float64), axis=1)
    c = np.pad(c, ((1, 0), (1, 0)))
    return (c[KH:, KW:] - c[:-KH, KW:] - c[KH:, :-KW]
            + c[:-KH, :-KW]).astype(np.float32)


def _prepare_in_maps(x, weight, bias):
    bf16 = _bf16()
    fp8 = _fp8()
    x = np.ascontiguousarray(x, dtype=np.float32)
    w = np.asarray(weight, dtype=np.float32)
    bias_v = float(np.asarray(bias, dtype=np.float32).reshape(-1)[0])

    w_hi = w.astype(fp8).astype(np.float32)
    w_lo_f = w - w_hi
    w_lo = w_lo_f.astype(fp8).astype(np.float32)

    x_pad = np.zeros((XR_PAD, XC_PAD), dtype=np.float32)
    x_pad[:H, :W] = x
    xq_pad = x_pad.astype(fp8)
    xq_f = xq_pad.astype(np.float32)

    # Host correction: mean(w)*boxsum(x - xq)  +  exact-residual column-7
    # vertical conv on xq  +  bias.
    x_lo = x_pad - xq_f
    corr = w.mean() * _boxsum15(x_lo)
    lam7 = (w[:, 7] - w_hi[:, 7]).astype(np.float32)  # exact col-7 residual
    ho_pad, wo_pad = corr.shape
    for di in range(KH):
        if lam7[di] != 0.0:
            corr += lam7[di] * xq_f[di:di + ho_pad, 7:7 + wo_pad]
    corr += bias_v
    corr = corr.astype(bf16)

    wT = _weight_stack(w_hi, w_lo).astype(fp8)

    in_maps = []
    for core in range(NCORES):
        c, r = core // HSH, core % HSH
        xs = xq_pad[r * B:r * B + BIN, c * C:c * C + XSW]
        cs = corr[r * B:r * B + B, c * C:c * C + C]
        in_maps.append({"xs": np.ascontiguousarray(xs), "wT": wT,
                        "corr": np.ascontiguousarray(cs)})
    return in_maps


def kernel(x: np.ndarray, weight: np.ndarray, bias: np.ndarray) -> np.ndarray:
    from concourse.bass_utils import run_bass_kernel_spmd

    if "nc" not in _CACHE:
        _CACHE["nc"] = _build_nc()
    nc = _CACHE["nc"]

    in_maps = _prepare_in_maps(x, weight, bias)
    res = run_bass_kernel_spmd(nc, in_maps, core_ids=list(range(NCORES)))

    out = np.empty((HO, WO), dtype=np.float32)
    for core in range(NCORES):
        c, r = core // HSH, core % HSH
        r0, r1 = r * B, min(r * B + B, HO)
        c0, c1 = c * C, min(c * C + C, WO)
        ys = res.results[core]["ys"]
        out[r0:r1, c0:c1] = ys[: r1 - r0, : c1 - c0].astype(np.float32)
    return out


# revision 5
# speedup vs baseline: 3.0611x; 2.3092x over previous
"""Trainium2 Bass kernel: 4096x4096 fp32 'valid' cross-correlation with a 15x15
kernel, plus scalar bias.

Strategy (v2: fp8 DoubleRow)
----------------------------
- Shard the output 2x4 across 8 NeuronCores: 2 W-stripes of 2048 cols x 4
  H-bands of 1026 rows (tails trimmed on the host). Per core, 9 h-chunks of
  114 output rows: a banded-Toeplitz stationary (T[k, m] = wcol[k-m])
  contracts 128 input rows against 114 output rows.
- fp8e4m3 operands with perf_mode=DoubleRow: the PE virtualizes to 128x256
  (2 fp8 weights/cell, 2 MACs/cycle, 0.5 cycles per moving column). The two
  interleave planes carry the SAME image rows at column shifts (0, +8), so
  one pass computes TWO column-taps (t, t+8). The 15x15 kernel is split
  w = w_hi + w_lo (both e4m3; residual ~1e-3) giving 29 tap-slots on device
  (w_lo column 7 is folded into the host correction) = 15 DoubleRow passes
  per chunk vs 15 bf16 passes at 1 cycle/column in v1 -> ~2x PE time.
- Accuracy (rel-err budget 2e-2): device error is dominated by fp8
  quantization of x. The dominant component is mean(w) * boxsum(x - xq),
  which the host computes exactly via prefix sums and uploads as a bf16
  correction field C (bias folded in); the drain adds C on the DVE. The
  remaining variance-floor error measures ~1.4e-2 rel.
- Input DMAs on the SP ring (nc.sync), C loads on the GPSIMD ring, output
  stores on the ACT ring (nc.scalar) so the streams don't share one FIFO.
"""

import numpy as np

H, W = 4096, 4096
KH, KW = 15, 15
HO, WO = H - KH + 1, W - KW + 1  # 4082, 4082
NCORES = 8
WSH, HSH = 2, 4          # core grid: 2 W-stripes x 4 H-bands
C = 2048                 # output cols per stripe
MCH = 114                # output rows per h-chunk (114 + 14 = 128 = K)
NCHUNK = 9               # chunks per band
B = NCHUNK * MCH         # output rows per band = 1026
BIN = B + KH - 1         # input rows per band = 1040
NBLK = C // 512          # 512-col psum blocks per chunk
CW = 2064                # moving-tile plane width (mult of 16 for DoubleRow AP)
XSW = 2080               # per-core input dram width (plane1 needs cols 8..2071)
XR_PAD = HSH * B + KH - 1   # padded input rows = 4118
XC_PAD = WSH * C + XSW - C  # padded input cols = 2048 + 2080 = 4128
NPASS = 15
# pass p -> (base column shift, plane0 tap source, plane1 tap source)
# sources: ('hi', col) / ('lo', col) / None; plane1 reads at base+8.
PASS_TABLE = (
    [(t, ("hi", t), ("hi", t + 8)) for t in range(7)]
    + [(t, ("lo", t), ("lo", t + 8)) for t in range(7)]
    + [(7, ("hi", 7), None)]
)

_CACHE = {}


def _bf16():
    import ml_dtypes
    return ml_dtypes.bfloat16


def _fp8():
    import ml_dtypes
    return ml_dtypes.float8_e4m3


def _enable_ldw_opt():
    """Flip walrus --enable-ldw-opt to true (dedupes identical consecutive
    weight loads, which the pass-outer/block-inner schedule produces)."""
    import concourse.bass_utils as bu
    if getattr(bu.run_command, "_ldw_patched", False):
        return
    orig = bu.run_command

    def patched(argv, **kw):
        argv = ["--enable-ldw-opt=true" if a == "--enable-ldw-opt=false" else a
                for a in argv]
        return orig(argv, **kw)

    patched._ldw_patched = True
    bu.run_command = patched


def _build_nc(reps: int = 1, hw_loop: bool = False, ldw_opt: bool = False,
              probe_same_w: bool = False,
              parts: tuple = ("in", "mm", "drain", "out")):
    import concourse.bacc as bacc
    import concourse.mybir as mybir
    from concourse.tile import TileContext

    if ldw_opt:
        _enable_ldw_opt()
    parts = set(parts)
    f32 = mybir.dt.float32
    bf16 = mybir.dt.bfloat16
    fp8 = mybir.dt.float8e4

    nc = bacc.Bacc("TRN2", debug=False, num_devices=NCORES)
    xs_d = nc.dram_tensor("xs", [BIN, XSW], fp8, kind="ExternalInput")
    wT_d = nc.dram_tensor("wT", [128, NPASS, 2, 128], fp8, kind="ExternalInput")
    corr_d = nc.dram_tensor("corr", [B, C], bf16, kind="ExternalInput")
    ys_d = nc.dram_tensor("ys", [B, C], bf16, kind="ExternalOutput")

    with TileContext(nc) as tc:
        with (
            tc.tile_pool(name="xp", bufs=2) as xp,
            tc.tile_pool(name="wp", bufs=1) as wp,
            tc.tile_pool(name="cp", bufs=2) as cp,
            tc.tile_pool(name="op", bufs=3) as op,
            tc.tile_pool(name="pp", bufs=2, space="PSUM") as pp,
        ):
            w_t = wp.tile([128, NPASS, 2, 128], fp8)
            nc.sync.dma_start(w_t[:, :, :, :], wT_d[:, :, :, :])

            x_s = c_s = o_s = None
            if "mm" in parts and "in" not in parts:
                x_s = wp.tile([128, 2, CW], fp8)
                nc.sync.dma_start(x_s[:, 0, :], xs_d[0:128, 0:CW])
                nc.sync.dma_start(x_s[:, 1, :], xs_d[0:128, 8:8 + CW])
                c_s = wp.tile([MCH, C], bf16)
                nc.gpsimd.dma_start(c_s[:, :], corr_d[0:MCH, :])
            if "out" in parts and "drain" not in parts:
                o_s = wp.tile([MCH, C], bf16)
                nc.vector.memset(o_s[:, :], 0.0)

            def rep_body(_i=None):
                for ci in range(NCHUNK):
                    m0 = ci * MCH
                    if "in" in parts:
                        x_b = xp.tile([128, 2, CW], fp8, name="x_b")
                        nc.sync.dma_start(x_b[:, 0, :],
                                          xs_d[m0:m0 + 128, 0:CW])
                        nc.sync.dma_start(x_b[:, 1, :],
                                          xs_d[m0:m0 + 128, 8:8 + CW])
                        c_t = cp.tile([MCH, C], bf16, name="c_t")
                        nc.gpsimd.dma_start(c_t[:, :],
                                            corr_d[m0:m0 + MCH, :])
                    else:
                        x_b, c_t = x_s, c_s
                    if "drain" in parts:
                        o = op.tile([MCH, C], bf16, name="o")
                    else:
                        o = o_s
                    if "mm" in parts:
                        pss = [pp.tile([128, 512], f32, name=f"ps{b}")
                               for b in range(NBLK)]
                        for p, (base, _s0, _s1) in enumerate(PASS_TABLE):
                            for blk in range(NBLK):
                                j0 = blk * 512 + base
                                nc.tensor.matmul(
                                    pss[blk][:, :],
                                    w_t[:, 0 if probe_same_w else p, :, :],
                                    x_b[:, :, j0:j0 + 512],
                                    start=(p == 0),
                                    stop=(p == NPASS - 1),
                                    perf_mode=mybir.MatmulPerfMode.DoubleRow,
                                    skip_group_check=True,
                                )
                        if "drain" in parts:
                            for blk in range(NBLK):
                                nc.vector.scalar_tensor_tensor(
                                    o[:, blk * 512:(blk + 1) * 512],
                                    pss[blk][0:MCH, :],
                                    1.0,
                                    c_t[:, blk * 512:(blk + 1) * 512],
                                    mybir.AluOpType.mult,
                                    mybir.AluOpType.add,
                                )
                    if "out" in parts:
                        nc.scalar.dma_start(ys_d[m0:m0 + MCH, :], o[:, :])

            if hw_loop and reps > 1:
                tc.For_i_unrolled(0, reps, 1, rep_body, max_unroll=8)
            else:
                for _rep in range(reps):
                    rep_body()

    nc.compile()
    return nc


def _toeplitz(col: np.ndarray) -> np.ndarray:
    """T[k, m] = col[k-m] for 0 <= k-m < KH (m < MCH; cols MCH..127 zero)."""
    T = np.zeros((128, 128), dtype=np.float32)
    for di in range(KH):
        for m in range(MCH):
            T[m + di, m] = col[di]
    return T


def _weight_stack(w_hi: np.ndarray, w_lo: np.ndarray) -> np.ndarray:
    """wT[k, pass, plane, m] per PASS_TABLE."""
    src = {"hi": w_hi, "lo": w_lo}
    wT = np.zeros((128, NPASS, 2, 128), dtype=np.float32)
    for p, (_base, s0, s1) in enumerate(PASS_TABLE):
        for plane, s in enumerate((s0, s1)):
            if s is not None:
                wT[:, p, plane, :] = _toeplitz(src[s[0]][:, s[1]])
    return wT


def _boxsum15(a: np.ndarray) -> np.ndarray:
    """Valid 15x15 box sum (fp64 prefix sums)."""
    c = np.cumsum(np.cumsum(a, axis=0, dtype=np.float64), axis=1)
    c = np.pad(c, ((1, 0), (1, 0)))
    return (c[KH:, KW:] - c[:-KH, KW:] - c[KH:, :-KW]
            + c[:-KH, :-KW]).astype(np.float32)


def _prepare_in_maps(x, weight, bias):
    bf16 = _bf16()
    fp8 = _fp8()
    x = np.ascontiguousarray(x, dtype=np.float32)
    w = np.asarray(weight, dtype=np.float32)
    bias_v = float(np.asarray(bias, dtype=np.float32).reshape(-1)[0])

    w_hi = w.astype(fp8).astype(np.float32)
    w_lo_f = w - w_hi
    w_lo = w_lo_f.astype(fp8).astype(np.float32)

    x_pad = np.zeros((XR_PAD, XC_PAD), dtype=np.float32)
    x_pad[:H, :W] = x
    xq_pad = x_pad.astype(fp8)
    xq_f = xq_pad.astype(np.float32)

    # Host correction: mean(w)*boxsum(x - xq)  +  exact-residual column-7
    # vertical conv on xq  +  bias.
    x_lo = x_pad - xq_f
    corr = w.mean() * _boxsum15(x_lo)
    lam7 = (w[:, 7] - w_hi[:, 7]).astype(np.float32)  # exact col-7 residual
    ho_pad, wo_pad = corr.shape
    for di in range(KH):
        if lam7[di] != 0.0:
            corr += lam7[di] * xq_f[di:di + ho_pad, 7:7 + wo_pad]
    corr += bias_v
    corr = corr.astype(bf16)

    wT = _weight_stack(w_hi, w_lo).astype(fp8)

    in_maps = []
    for core in range(NCORES):
        c, r = core // HSH, core % HSH
        xs = xq_pad[r * B:r * B + BIN, c * C:c * C + XSW]
        cs = corr[r * B:r * B + B, c * C:c * C + C]
        in_maps.append({"xs": np.ascontiguousarray(xs), "wT": wT,
                        "corr": np.ascontiguousarray(cs)})
    return in_maps


def kernel(x: np.ndarray, weight: np.ndarray, bias: np.ndarray) -> np.ndarray:
    from concourse.bass_utils import run_bass_kernel_spmd

    if "nc" not in _CACHE:
        _CACHE["nc"] = _build_nc()
    nc = _CACHE["nc"]

    in_maps = _prepare_in_maps(x, weight, bias)
    res = run_bass_kernel_spmd(nc, in_maps, core_ids=list(range(NCORES)))

    out = np.empty((HO, WO), dtype=np.float32)
    for core in range(NCORES):
        c, r = core // HSH, core % HSH
        r0, r1 = r * B, min(r * B + B, HO)
        c0, c1 = c * C, min(c * C + C, WO)
        ys = res.results[core]["ys"]
        out[r0:r1, c0:c1] = ys[: r1 - r0, : c1 - c0].astype(np.float32)
    return out
